# revision 35
# baseline (speedup 1.0000x reference)
"""Trainium2 Bass kernel for a Bahdanau-attention decoder step.

Computes, for B=16, L=4096, A=512, H=512:
    dec  = concat(h, c) @ W.T + b                      # [B, A]
    e    = sum_a v[a] * tanh(feat[b,l,a] + dec[b,a])   # [B, L]
    attn = softmax(e) * mask, renormalized             # [B, L]
    ctx  = sum_l attn[b,l] * state[b,l,a]              # [B, A]

Sharding: data-parallel over batch B across 8 NeuronCores (2 rows/core).
The tiny dec projection (16x1024 @ 1024x512) is done host-side; dec, v and
mask are passed in device-friendly layouts so the kernel streams the two
134MB tensors exactly once each (memory-bound target).

Device dataflow per core (2 batch rows, 4 L-groups of 1024 per row, each
group = 8 subtiles of 128 L x 512 A, natural layout: L on partitions):
  DMA   : 2MB contiguous loads of feat/state groups
  DVE   : X = feat + dec_bcast (dec broadcast materialized once per batch
          via a K=1 PE outer product)
  ACT   : T = tanh(X)
  DVE   : prod = T * v_bcast (in-place over X)
  ACT   : e[:,u] = accum_out of an Identity activation over prod
          -- e lands directly in column layout [128L, 1]
  ACT   : w = exp(e)  (no max-subtraction: |e| <~ 40 so fp32 exp is safe)
  DVE   : W_sb[:, cols] = w * maskT
  PE    : ctx += W_sb[:,u].T @ state subtile
  finale: denom via transpose+reduce, reciprocal, scale, transpose attn out.

This walrus build accepts only ONE semaphore wait per instruction, so the
trace order and tile reuse are arranged to keep every instruction at one
cross-engine wait (see _split_multiwaits for the general fallback).
"""

import os
import sys

import numpy as np

sys.path.insert(0, "/opt/trn_rl_repo")

B, L, A, H = 16, 4096, 512, 512
NCORES = 8
BPC = B // NCORES      # batch rows per core
G = 4                  # L-groups per batch row (1024 L each, 2MB per DMA)
J = 8                  # 128-L subtiles per group
U = G * J              # 32 subtiles per batch row
P = 128

_CACHE = {}
LAST_RESULTS = None    # BassKernelResults of the most recent run (for test.py)
TRACE = False


def _build_nc():
    import concourse.bass as bass
    import concourse.tile as tile
    from concourse import masks, mybir
    from concourse.tile import add_dep_helper

    f32 = mybir.dt.float32
    Act = mybir.ActivationFunctionType

    nc = bass.Bass()

    feat = nc.dram_tensor("feat", [BPC, L, A], f32, kind="ExternalInput")
    state = nc.dram_tensor("state", [BPC, L, A], f32, kind="ExternalInput")
    dec = nc.dram_tensor("dec", [1, BPC * A], f32, kind="ExternalInput")
    vrow = nc.dram_tensor("vrow", [1, A], f32, kind="ExternalInput")
    maskT = nc.dram_tensor("maskT", [P, BPC * U], f32, kind="ExternalInput")
    ctx_out = nc.dram_tensor("ctx", [BPC, A], f32, kind="ExternalOutput")
    attn_out = nc.dram_tensor("attn", [BPC, L], f32, kind="ExternalOutput")

    # group g holds L rows 1024g..1024g+1023; partition p carries row
    # 1024g + 128j + p
    featR = feat.rearrange("b (g j p) a -> b g p j a", g=G, j=J, p=P)
    stateR = state.rearrange("b (g j p) a -> b g p j a", g=G, j=J, p=P)
    attnR = attn_out.rearrange("b (u q) -> b u q", u=U)

    with tile.TileContext(nc) as tc:
        with (
            tc.tile_pool(name="consts", bufs=1) as consts,
            tc.tile_pool(name="fpool", bufs=2) as fpool,
            tc.tile_pool(name="spool", bufs=3) as spool,
            tc.tile_pool(name="xsp", bufs=4) as xsp,
            tc.tile_pool(name="tanhp", bufs=4) as tanhp,
            tc.tile_pool(name="wp", bufs=4) as wp,
            tc.tile_pool(name="wsbp", bufs=2) as wsbp,
            tc.tile_pool(name="finsb", bufs=2) as finsb,
            tc.tile_pool(name="bcps", bufs=2, space="PSUM") as bcps,
            tc.tile_pool(name="ctxps", bufs=1, space="PSUM") as ctxps,
            tc.tile_pool(name="finps", bufs=1, space="PSUM") as finps,
        ):
            ident = consts.tile([P, P], f32)
            masks.make_identity(nc, ident[:])
            ones_col = consts.tile([P, 1], f32)
            nc.gpsimd.memset(ones_col[:], 1.0)
            ones_row = consts.tile([1, P], f32)
            nc.gpsimd.memset(ones_row[:], 1.0)

            dec_sb = consts.tile([1, BPC * A], f32)
            nc.sync.dma_start(out=dec_sb[:], in_=dec[:])
            v_sb = consts.tile([1, A], f32)
            nc.sync.dma_start(out=v_sb[:], in_=vrow[:])
            maskT_sb = consts.tile([P, BPC * U], f32)
            nc.sync.dma_start(out=maskT_sb[:], in_=maskT[:])

            # PSUM tiles, allocated once and reused across batch rows

            ctx_ps = ctxps.tile([1, A], f32, tag="ctx")
            fin_ps = finps.tile([P, 1 + 2 * P], f32, tag="fin")

            sc_d = consts.tile([P, 1], f32)
            nc.vector.tensor_copy(sc_d[:], maskT_sb[:, :1])   # DVE <- maskT

            # v broadcast to all partitions (x4 replicated for quad-wide
            # elementwise ops): K=1 outer product + ACT copies
            v_bc = consts.tile([P, 4, A], f32)
            vb_ps = bcps.tile([P, A], f32, tag="bc", name="vb")
            nc.tensor.matmul(vb_ps[:], ones_row[:], v_sb[:1, :], start=True, stop=True)
            for jj in range(4):
                nc.scalar.copy(v_bc[:, jj, :], vb_ps[:])

            # Cross-batch ordering anchors: the scheduler treats PSUM
            # accumulation groups as commutative, so overwrites need
            # explicit deps on the previous readers.
            prev_ctx_read = None
            prev_bc_read = None

            for b in range(BPC):
                W_sb = wsbp.tile([P, U], f32, tag="wsb")
                ecols = wsbp.tile([P, U], f32, tag="ecols")

                # dec broadcast for this batch row (own psum slot per row)
                bc_ps = bcps.tile([P, A], f32, tag="bc", name=f"bc{b}")
                nc.tensor.matmul(
                    bc_ps[:], ones_row[:], dec_sb[:1, b * A : (b + 1) * A],
                    start=True, stop=True,
                )
                dec_bc = consts.tile(
                    [P, 4, A], f32, tag=f"dec_bc{b}", name=f"dec_bc{b}"
                )
                for jj in range(4):
                    prev_bc_read = nc.scalar.copy(dec_bc[:, jj, :], bc_ps[:])

                for g in range(G):
                    F_g = fpool.tile([P, J, A], f32, tag="fg")
                    nc.sync.dma_start(out=F_g[:], in_=featR[b, g])
                    S_g = spool.tile([P, J, A], f32, tag="sg")
                    nc.sync.dma_start(out=S_g[:], in_=stateR[b, g])

                    # the very last group drains the pipeline, so use
                    # half-size units there to shorten the chain latency
                    nun = 4 if (b == BPC - 1 and g == G - 1) else 2
                    w_unit = J // nun
                    for q in range(nun):
                        u0 = J * g + w_unit * q
                        qi = 2 * g + q + 8 * b  # unit index (engine split)
                        Xs = xsp.tile([P, 4, A], f32, tag="xs")
                        nc.vector.tensor_add(
                            Xs[:, :w_unit, :],
                            F_g[:, w_unit * q : w_unit * (q + 1), :],
                            dec_bc[:, :w_unit, :],
                        )
                        T_q = tanhp.tile([P, 4, A], f32, tag="th")
                        nc.scalar.activation(
                            T_q[:, :w_unit, :], Xs[:, :w_unit, :], Act.Tanh
                        )
                        nc.vector.tensor_mul(
                            Xs[:, :w_unit, :], T_q[:, :w_unit, :],
                            v_bc[:, :w_unit, :],
                        )
                        if qi % 4 == 0:
                            # a quarter of the reductions on DVE (segmented
                            # reduce) to balance engines
                            nc.vector.tensor_reduce(
                                ecols[:, u0 : u0 + w_unit], Xs[:, :w_unit, :],
                                mybir.AxisListType.X, mybir.AluOpType.add,
                            )
                        else:
                            for jj in range(w_unit):
                                nc.scalar.activation(
                                    Xs[:, jj, :], Xs[:, jj, :], Act.Identity,
                                    accum_out=ecols[:, u0 + jj : u0 + jj + 1],
                                )

                    # exp + mask for this group's 8 columns
                    c0 = J * g
                    wexp = wp.tile([P, J], f32, tag="wexp")
                    nc.scalar.activation(wexp[:], ecols[:, c0 : c0 + J], Act.Exp)
                    nc.vector.tensor_mul(
                        W_sb[:, c0 : c0 + J],
                        wexp[:],
                        maskT_sb[:, U * b + c0 : U * b + c0 + J],
                    )

                    for j in range(J):
                        u = J * g + j
                        mm = nc.tensor.matmul(
                            ctx_ps[:],
                            W_sb[:, u : u + 1],
                            S_g[:, j, :],
                            start=(u == 0),
                            stop=(u == U - 1),
                        )
                        if u == 0 and prev_ctx_read is not None:
                            add_dep_helper(mm.ins, prev_ctx_read.ins,
                                           reason="ctx WAR vs previous read")

                # ---- batch-row finalization ----
                # fin_ps regions: col 0 = reciprocal broadcast, cols 1..128 =
                # rowsum transpose (also scratch), cols 129..256 = attn
                # transpose.
                rowsum = finsb.tile([P, 1], f32, tag="rowsum")
                nc.vector.tensor_reduce(
                    rowsum[:], W_sb[:], mybir.AxisListType.X, mybir.AluOpType.add
                )
                nc.tensor.transpose(fin_ps[:1, 1 : 1 + P], rowsum[:], ident[:])
                denom = finsb.tile([1, 1], f32, tag="denom")
                nc.vector.tensor_reduce(
                    denom[:], fin_ps[:1, 1 : 1 + P], mybir.AxisListType.X,
                    mybir.AluOpType.add,
                )
                recip = finsb.tile([1, 1], f32, tag="recip")
                nc.vector.reciprocal(recip[:], denom[:])

                # broadcast 1/denom to all partitions via K=1 outer product
                nc.tensor.matmul(
                    fin_ps[:, 0:1], ones_row[:], recip[:], start=True, stop=True
                )
                rb = finsb.tile([P, 1], f32, tag="rb")
                nc.vector.tensor_copy(rb[:], fin_ps[:, 0:1])

                attn_scaled = finsb.tile([P, U], f32, tag="ascaled")
                nc.vector.tensor_scalar_mul(attn_scaled[:], W_sb[:], rb[:])

                nc.tensor.transpose(
                    fin_ps[:U, 1 + P : 1 + 2 * P], attn_scaled[:], ident[:]
                )
                nc.scalar.copy(sc_d[:1, :1], recip[:])  # ACT observes DVE recip
                attn_sb = finsb.tile([U, P], f32, tag="asb")
                nc.scalar.copy(attn_sb[:], fin_ps[:U, 1 + P : 1 + 2 * P])
                nc.sync.dma_start(out=attnR[b], in_=attn_sb[:])

                ctx_sb = finsb.tile([1, A], f32, tag="ctxsb")
                prev_ctx_read = nc.scalar.mul(ctx_sb[:], ctx_ps[:], recip[:1, :1])
                nc.sync.dma_start(out=ctx_out[b : b + 1, :], in_=ctx_sb[:])

    _split_multiwaits(nc)
    return nc


def _split_multiwaits(nc):
    """Walrus in this toolchain accepts only ONE semaphore wait per
    instruction. Tile occasionally emits more (data dep + hazard dep on
    another engine). Splitting is semantics-preserving: engine streams
    execute in order, so hoisting extra waits onto same-engine NoOps
    immediately before the instruction blocks identically."""
    import concourse.mybir as mybir

    n_split = 0
    for fn in nc.m.functions:
        for blk in fn.blocks:
            insts = blk.instructions
            i = 0
            while i < len(insts):
                inst = insts[i]
                si = getattr(inst, "sync_info", None)
                eng = getattr(inst, "engine", None)
                engname = str(eng).split(".")[-1] if eng is not None else ""
                if (
                    si is not None
                    and si.on_wait
                    and len(si.on_wait) > 1
                    and engname in ("Activation", "PE", "DVE", "Pool", "SP")
                    and type(inst).__name__ != "InstISA"
                ):
                    waits = list(si.on_wait)
                    for k, w in enumerate(waits[:-1]):
                        nop = mybir.InstNoOp(name=f"{inst.name}-ws{k}", engine=eng)
                        nop.sync_info = mybir.SyncInfo(on_wait=[w], on_update=[])
                        insts.insert(i, nop)
                        i += 1
                    inst.sync_info = mybir.SyncInfo(
                        on_wait=[waits[-1]], on_update=list(si.on_update or [])
                    )
                    n_split += 1
                i += 1
    return nc


def _get_nc():
    if "nc" not in _CACHE:
        _CACHE["nc"] = _build_nc()
    return _CACHE["nc"]


def kernel(encoder_features, h, c, encoder_state, encoder_mask, v, W, b):
    global LAST_RESULTS
    from concourse.bass_utils import run_bass_kernel_spmd

    ef = np.ascontiguousarray(np.asarray(encoder_features, np.float32)).reshape(B, L, A)
    es = np.ascontiguousarray(np.asarray(encoder_state, np.float32)).reshape(B, L, A)
    h = np.asarray(h, np.float32)
    c = np.asarray(c, np.float32)
    mask = np.asarray(encoder_mask, np.float32)
    v = np.asarray(v, np.float32)
    W = np.asarray(W, np.float32)
    bb = np.asarray(b, np.float32)

    # dec = [h, c] @ W.T + b  (tiny: 16x1024 @ 1024x512)
    dec = np.concatenate([h, c], axis=1) @ W.T + bb  # [B, A]
    vr = np.ascontiguousarray(v.reshape(1, A))

    in_maps = []
    for k in range(NCORES):
        sl = slice(BPC * k, BPC * (k + 1))
        # maskT[p, U*b + u] = mask[b, 128u + p]
        maskT = np.ascontiguousarray(
            np.concatenate(
                [mask[BPC * k + i].reshape(U, P).T for i in range(BPC)], axis=1
            )
        )
        in_maps.append(
            {
                "feat": np.ascontiguousarray(ef[sl]),
                "state": np.ascontiguousarray(es[sl]),
                "dec": np.ascontiguousarray(dec[sl].reshape(1, BPC * A)),
                "vrow": vr,
                "maskT": maskT,
            }
        )

    nc = _get_nc()
    res = run_bass_kernel_spmd(
        nc,
        in_maps,
        core_ids=list(range(NCORES)),
        trace=TRACE or bool(int(os.environ.get("KERNEL_TRACE", "0"))),
    )
    LAST_RESULTS = res

    context = np.concatenate([r["ctx"] for r in res.results], axis=0)
    attn = np.concatenate([r["attn"] for r in res.results], axis=0)
    return context, attn


# revision 36
# speedup vs baseline: 1.0734x; 1.0734x over previous
"""Trainium2 Bass kernel for a Bahdanau-attention decoder step.

Computes, for B=16, L=4096, A=512, H=512:
    dec  = concat(h, c) @ W.T + b                      # [B, A]
    e    = sum_a v[a] * tanh(feat[b,l,a] + dec[b,a])   # [B, L]
    attn = softmax(e) * mask, renormalized             # [B, L]
    ctx  = sum_l attn[b,l] * state[b,l,a]              # [B, A]

Sharding: data-parallel over batch B across 8 NeuronCores (2 rows/core).
The tiny dec projection (16x1024 @ 1024x512) is done host-side; dec, v and
mask are passed in device-friendly layouts so the kernel streams the two
134MB tensors exactly once each (memory-bound target).

Device dataflow per core (2 batch rows, 4 L-groups of 1024 per row, each
group = 8 subtiles of 128 L x 512 A, natural layout: L on partitions):
  DMA   : 2MB contiguous loads of feat/state groups
  DVE   : X = feat + dec_bcast (dec broadcast materialized once per batch
          via a K=1 PE outer product)
  ACT   : T = tanh(X)
  DVE   : prod = T * v_bcast (in-place over X)
  ACT   : e[:,u] = accum_out of an Identity activation over prod
          -- e lands directly in column layout [128L, 1]
  ACT   : w = exp(e)  (no max-subtraction: |e| <~ 40 so fp32 exp is safe)
  DVE   : W_sb[:, cols] = w * maskT
  PE    : ctx += W_sb[:,u].T @ state subtile
  finale: denom via transpose+reduce, reciprocal, scale, transpose attn out.

This walrus build accepts only ONE semaphore wait per instruction, so the
trace order and tile reuse are arranged to keep every instruction at one
cross-engine wait (see _split_multiwaits for the general fallback).
"""

import os
import sys

import numpy as np

sys.path.insert(0, "/opt/trn_rl_repo")

B, L, A, H = 16, 4096, 512, 512
NCORES = 8
BPC = B // NCORES      # batch rows per core
G = 4                  # L-groups per batch row (1024 L each, 2MB per DMA)
J = 8                  # 128-L subtiles per group
U = G * J              # 32 subtiles per batch row
P = 128

_CACHE = {}
LAST_RESULTS = None    # BassKernelResults of the most recent run (for test.py)
TRACE = False


def _build_nc():
    import concourse.bass as bass
    import concourse.tile as tile
    from concourse import masks, mybir
    from concourse.tile import add_dep_helper

    f32 = mybir.dt.float32
    Act = mybir.ActivationFunctionType

    nc = bass.Bass()

    feat = nc.dram_tensor("feat", [BPC, L, A], f32, kind="ExternalInput")
    state = nc.dram_tensor("state", [BPC, L, A], f32, kind="ExternalInput")
    dec = nc.dram_tensor("dec", [1, BPC * A], f32, kind="ExternalInput")
    vrow = nc.dram_tensor("vrow", [1, A], f32, kind="ExternalInput")
    maskT = nc.dram_tensor("maskT", [P, BPC * U], f32, kind="ExternalInput")
    ctx_out = nc.dram_tensor("ctx", [BPC, A], f32, kind="ExternalOutput")
    attn_out = nc.dram_tensor("attn", [BPC, L], f32, kind="ExternalOutput")

    # group g holds L rows 1024g..1024g+1023; partition p carries row
    # 1024g + 128j + p
    featR = feat.rearrange("b (g j p) a -> b g p j a", g=G, j=J, p=P)
    stateR = state.rearrange("b (g j p) a -> b g p j a", g=G, j=J, p=P)
    attnR = attn_out.rearrange("b (u q) -> b u q", u=U)

    with tile.TileContext(nc) as tc:
        with (
            tc.tile_pool(name="consts", bufs=1) as consts,
            tc.tile_pool(name="fpool", bufs=2) as fpool,
            tc.tile_pool(name="spool", bufs=3) as spool,
            tc.tile_pool(name="xsp", bufs=4) as xsp,
            tc.tile_pool(name="tanhp", bufs=4) as tanhp,
            tc.tile_pool(name="wp", bufs=4) as wp,
            tc.tile_pool(name="wsbp", bufs=2) as wsbp,
            tc.tile_pool(name="finsb", bufs=2) as finsb,
            tc.tile_pool(name="bcps", bufs=2, space="PSUM") as bcps,
            tc.tile_pool(name="ctxps", bufs=1, space="PSUM") as ctxps,
            tc.tile_pool(name="finps", bufs=1, space="PSUM") as finps,
        ):
            ident = consts.tile([P, P], f32)
            masks.make_identity(nc, ident[:])
            ones_col = consts.tile([P, 1], f32)
            nc.gpsimd.memset(ones_col[:], 1.0)
            ones_row = consts.tile([1, P], f32)
            nc.gpsimd.memset(ones_row[:], 1.0)

            dec_sb = consts.tile([1, BPC * A], f32)
            nc.sync.dma_start(out=dec_sb[:], in_=dec[:])
            v_sb = consts.tile([1, A], f32)
            nc.sync.dma_start(out=v_sb[:], in_=vrow[:])
            maskT_sb = consts.tile([P, BPC * U], f32)
            nc.sync.dma_start(out=maskT_sb[:], in_=maskT[:])

            # PSUM tiles, allocated once and reused across batch rows

            ctx_ps = ctxps.tile([1, A], f32, tag="ctx")
            fin_ps = finps.tile([P, 1 + 2 * P], f32, tag="fin")

            sc_d = consts.tile([P, 1], f32)
            nc.vector.tensor_copy(sc_d[:], maskT_sb[:, :1])   # DVE <- maskT

            # v broadcast to all partitions (x4 replicated for quad-wide
            # elementwise ops): K=1 outer product + ACT copies
            v_bc = consts.tile([P, 4, A], f32)
            vb_ps = bcps.tile([P, A], f32, tag="bc", name="vb")
            nc.tensor.matmul(vb_ps[:], ones_row[:], v_sb[:1, :], start=True, stop=True)
            for jj in range(4):
                nc.scalar.copy(v_bc[:, jj, :], vb_ps[:])

            # Cross-batch ordering anchors: the scheduler treats PSUM
            # accumulation groups as commutative, so overwrites need
            # explicit deps on the previous readers.
            prev_ctx_read = None
            prev_bc_read = None

            for b in range(BPC):
                W_sb = wsbp.tile([P, U], f32, tag="wsb")
                ecols = wsbp.tile([P, U], f32, tag="ecols")

                # dec broadcast for this batch row (own psum slot per row)
                bc_ps = bcps.tile([P, A], f32, tag="bc", name=f"bc{b}")
                nc.tensor.matmul(
                    bc_ps[:], ones_row[:], dec_sb[:1, b * A : (b + 1) * A],
                    start=True, stop=True,
                )
                dec_bc = consts.tile(
                    [P, 4, A], f32, tag=f"dec_bc{b}", name=f"dec_bc{b}"
                )
                for jj in range(4):
                    prev_bc_read = nc.scalar.copy(dec_bc[:, jj, :], bc_ps[:])

                for g in range(G):
                    F_g = fpool.tile([P, J, A], f32, tag="fg")
                    nc.sync.dma_start(out=F_g[:], in_=featR[b, g])
                    S_g = spool.tile([P, J, A], f32, tag="sg")
                    nc.sync.dma_start(out=S_g[:], in_=stateR[b, g])

                    for q in range(2):
                        u0 = J * g + 4 * q
                        qi = 2 * g + q + 8 * b  # global quad index
                        Xs = xsp.tile([P, 4, A], f32, tag="xs")
                        nc.vector.tensor_add(
                            Xs[:], F_g[:, 4 * q : 4 * q + 4, :], dec_bc[:]
                        )
                        T_q = tanhp.tile([P, 4, A], f32, tag="th")
                        nc.scalar.activation(T_q[:], Xs[:], Act.Tanh)
                        nc.vector.tensor_mul(Xs[:], T_q[:], v_bc[:])
                        if qi % 4 == 0:
                            # a quarter of the reductions on DVE (segmented
                            # quad reduce) to balance engines
                            nc.vector.tensor_reduce(
                                ecols[:, u0 : u0 + 4], Xs[:],
                                mybir.AxisListType.X, mybir.AluOpType.add,
                            )
                        else:
                            for jj in range(4):
                                nc.scalar.activation(
                                    Xs[:, jj, :], Xs[:, jj, :], Act.Identity,
                                    accum_out=ecols[:, u0 + jj : u0 + jj + 1],
                                )

                    # exp + mask for this group's 8 columns
                    c0 = J * g
                    wexp = wp.tile([P, J], f32, tag="wexp")
                    nc.scalar.activation(wexp[:], ecols[:, c0 : c0 + J], Act.Exp)
                    nc.vector.tensor_mul(
                        W_sb[:, c0 : c0 + J],
                        wexp[:],
                        maskT_sb[:, U * b + c0 : U * b + c0 + J],
                    )

                    for j in range(J):
                        u = J * g + j
                        mm = nc.tensor.matmul(
                            ctx_ps[:],
                            W_sb[:, u : u + 1],
                            S_g[:, j, :],
                            start=(u == 0),
                            stop=(u == U - 1),
                        )
                        if u == 0 and prev_ctx_read is not None:
                            add_dep_helper(mm.ins, prev_ctx_read.ins,
                                           reason="ctx WAR vs previous read")

                # ---- batch-row finalization ----
                # fin_ps regions: col 0 = reciprocal broadcast, cols 1..128 =
                # rowsum transpose (also scratch), cols 129..256 = attn
                # transpose.
                rowsum = finsb.tile([P, 1], f32, tag="rowsum")
                nc.vector.tensor_reduce(
                    rowsum[:], W_sb[:], mybir.AxisListType.X, mybir.AluOpType.add
                )
                nc.tensor.transpose(fin_ps[:1, 1 : 1 + P], rowsum[:], ident[:])
                denom = finsb.tile([1, 1], f32, tag="denom")
                nc.vector.tensor_reduce(
                    denom[:], fin_ps[:1, 1 : 1 + P], mybir.AxisListType.X,
                    mybir.AluOpType.add,
                )
                recip = finsb.tile([1, 1], f32, tag="recip")
                nc.vector.reciprocal(recip[:], denom[:])

                # broadcast 1/denom to all partitions via K=1 outer product
                nc.tensor.matmul(
                    fin_ps[:, 0:1], ones_row[:], recip[:], start=True, stop=True
                )
                rb = finsb.tile([P, 1], f32, tag="rb")
                nc.vector.tensor_copy(rb[:], fin_ps[:, 0:1])

                attn_scaled = finsb.tile([P, U], f32, tag="ascaled")
                nc.vector.tensor_scalar_mul(attn_scaled[:], W_sb[:], rb[:])

                nc.tensor.transpose(
                    fin_ps[:U, 1 + P : 1 + 2 * P], attn_scaled[:], ident[:]
                )
                nc.scalar.copy(sc_d[:1, :1], recip[:])  # ACT observes DVE recip
                attn_sb = finsb.tile([U, P], f32, tag="asb")
                nc.scalar.copy(attn_sb[:], fin_ps[:U, 1 + P : 1 + 2 * P])
                nc.sync.dma_start(out=attnR[b], in_=attn_sb[:])

                ctx_sb = finsb.tile([1, A], f32, tag="ctxsb")
                prev_ctx_read = nc.scalar.mul(ctx_sb[:], ctx_ps[:], recip[:1, :1])
                nc.sync.dma_start(out=ctx_out[b : b + 1, :], in_=ctx_sb[:])

    _split_multiwaits(nc)
    return nc


def _split_multiwaits(nc):
    """Walrus in this toolchain accepts only ONE semaphore wait per
    instruction. Tile occasionally emits more (data dep + hazard dep on
    another engine). Splitting is semantics-preserving: engine streams
    execute in order, so hoisting extra waits onto same-engine NoOps
    immediately before the instruction blocks identically."""
    import concourse.mybir as mybir

    n_split = 0
    for fn in nc.m.functions:
        for blk in fn.blocks:
            insts = blk.instructions
            i = 0
            while i < len(insts):
                inst = insts[i]
                si = getattr(inst, "sync_info", None)
                eng = getattr(inst, "engine", None)
                engname = str(eng).split(".")[-1] if eng is not None else ""
                if (
                    si is not None
                    and si.on_wait
                    and len(si.on_wait) > 1
                    and engname in ("Activation", "PE", "DVE", "Pool", "SP")
                    and type(inst).__name__ != "InstISA"
                ):
                    waits = list(si.on_wait)
                    for k, w in enumerate(waits[:-1]):
                        nop = mybir.InstNoOp(name=f"{inst.name}-ws{k}", engine=eng)
                        nop.sync_info = mybir.SyncInfo(on_wait=[w], on_update=[])
                        insts.insert(i, nop)
                        i += 1
                    inst.sync_info = mybir.SyncInfo(
                        on_wait=[waits[-1]], on_update=list(si.on_update or [])
                    )
                    n_split += 1
                i += 1
    return nc


def _get_nc():
    if "nc" not in _CACHE:
        _CACHE["nc"] = _build_nc()
    return _CACHE["nc"]


def kernel(encoder_features, h, c, encoder_state, encoder_mask, v, W, b):
    global LAST_RESULTS
    from concourse.bass_utils import run_bass_kernel_spmd

    ef = np.ascontiguousarray(np.asarray(encoder_features, np.float32)).reshape(B, L, A)
    es = np.ascontiguousarray(np.asarray(encoder_state, np.float32)).reshape(B, L, A)
    h = np.asarray(h, np.float32)
    c = np.asarray(c, np.float32)
    mask = np.asarray(encoder_mask, np.float32)
    v = np.asarray(v, np.float32)
    W = np.asarray(W, np.float32)
    bb = np.asarray(b, np.float32)

    # dec = [h, c] @ W.T + b  (tiny: 16x1024 @ 1024x512)
    dec = np.concatenate([h, c], axis=1) @ W.T + bb  # [B, A]
    vr = np.ascontiguousarray(v.reshape(1, A))

    in_maps = []
    for k in range(NCORES):
        sl = slice(BPC * k, BPC * (k + 1))
        # maskT[p, U*b + u] = mask[b, 128u + p]
        maskT = np.ascontiguousarray(
            np.concatenate(
                [mask[BPC * k + i].reshape(U, P).T for i in range(BPC)], axis=1
            )
        )
        in_maps.append(
            {
                "feat": np.ascontiguousarray(ef[sl]),
                "state": np.ascontiguousarray(es[sl]),
                "dec": np.ascontiguousarray(dec[sl].reshape(1, BPC * A)),
                "vrow": vr,
                "maskT": maskT,
            }
        )

    nc = _get_nc()
    res = run_bass_kernel_spmd(
        nc,
        in_maps,
        core_ids=list(range(NCORES)),
        trace=TRACE or bool(int(os.environ.get("KERNEL_TRACE", "0"))),
    )
    LAST_RESULTS = res

    context = np.concatenate([r["ctx"] for r in res.results], axis=0)
    attn = np.concatenate([r["attn"] for r in res.results], axis=0)
    return context, attn
